# revision 8
# baseline (speedup 1.0000x reference)
"""Trainium2 Bass kernel for pairwise-similarity distillation loss.

Reference, per image i of the folded batch (B*L = 8 images, each
[C=32, HW=4096]):

    That = T / ||T||_channels;  Shat likewise
    loss = sum_i || That_i^T That_i - Shat_i^T Shat_i ||_F^2 / (HW^2 * B * L)

The HW x HW Gram matrices are never materialized.  With V = [That; Shat]
(64 x HW) and J = diag(+1 x32, -1 x32):

    || G_T - G_S ||_F^2 = tr(J M J M),   M = V V^T  (64 x 64)

so the kernel is memory-bound: each core reads one image pair and emits a
64-float partial row; the host applies the column signs / scale and sums
across cores (the "all-reduce" of the sharding hint).

Sharding: data-parallel over the 8 images, one per NeuronCore.

Host-side prep (layout + precision marshaling only, no math):
  - V is transposed to position-major chunks [128 pos, 64 chan] so no PE
    transposes are needed on device and every DMA row is contiguous.
  - data is shipped fp16 (the pipeline tolerance is 2e-2; the fp16
    pipeline's end-to-end error is ~4e-5), halving HBM traffic and
    enabling the DVE 2x mode + 1-cycle/row PE matmuls.

Per-core dataflow (Tile framework schedules all sync):
  - 3-4 DMA waves alternate the SP/ACT HWDGE queues so descriptor-gen
    (625ns/DMA, single slot) pipelines under the serialized transfers.
  - per wave: square (ACT or DVE) -> channel-norm reduce (DVE or Pool)
    -> paired reciprocal (DVE, writes each 1/n^2 twice) -> sqrt (ACT)
    -> normalize mult (DVE; the duplicated-r view keeps the last AP dim
    packed, which unlocks the DVE 2x fp16 mode) -> PE Gram accumulate.
  - epilogue: msq = M*M (ACT, from PSUM), PE collapse with the +-1 sign
    vector -> [1, 64] row, copy to SBUF (ACT), DMA out.
"""

import numpy as np
from contextlib import ExitStack

import concourse.bass as bass
import concourse.tile as tile
from concourse import bacc, mybir
from concourse.bass_utils import run_bass_kernel_spmd

F16 = mybir.dt.float16
F32 = mybir.dt.float32

N_CORES = 8
B, L, C, H, W = 2, 4, 32, 64, 64
HW = H * W            # 4096
C2 = 2 * C            # 64: T channels stacked on S channels
NCHUNK = HW // 128    # 32 chunks of [128 pos, 64 chan]
SCALE = 1.0 / (float(HW) * float(HW) * float(B) * float(L))
CPRE = 1              # const prefix col: [sgn]

# (n_chunks, sq_engine, red_engine) per DMA wave; sq: "act"/"dve",
# reduce: "dve" (single grouped tensor_reduce), "fold" (DVE fp16 2x add
# tree), "pfold" (the first, largest fold level on Pool, rest on DVE).
# DMA queues alternate SP / ACT.
WAVES = [
    (12, "act", "fold"),
    (12, "act", "fold"),
    (8, "dve", "dve"),
]


def _emit(tc: tile.TileContext, out_ap, slab, waves):
    nc = tc.nc
    assert sum(n for n, _, _ in waves) == NCHUNK
    # The all-fp16 pipeline was validated end-to-end on the host: ~4e-5 rel
    # error vs the 2e-2 gate (see module docstring).
    with ExitStack() as ctx:
        ctx.enter_context(nc.allow_low_precision(reason="fp16 pipeline, ~4e-5 err"))
        data_pool = ctx.enter_context(tc.tile_pool(name="data", bufs=1))
        work = ctx.enter_context(tc.tile_pool(name="work", bufs=len(waves)))
        acc_pool = ctx.enter_context(tc.tile_pool(name="acc", bufs=1, space="PSUM"))

        # Warm the single ACT table while the first DMA is in flight:
        # abs_reciprocal_sqrt_and_small contains ars, square, and copy, so
        # every ACT op in this kernel shares one LoadActFuncSet.
        ARS = mybir.ActivationFunctionType.Abs_reciprocal_sqrt
        warm_in = work.tile([1, 2], F16, tag="warm_in")
        nc.vector.memset(warm_in[:], 1.0)
        warm_out = work.tile([1, 2], F16, tag="warm_out")
        nc.scalar.activation(warm_out[:, 0:1], warm_in[:, 0:1], ARS)
        nc.scalar.square(warm_out[:, 1:2], warm_in[:, 1:2])

        # Whole-slab SBUF tile; each wave DMAs a contiguous column range.
        slab_sb = data_pool.tile([128, CPRE + NCHUNK * C2], F16, tag="slab")
        sgn = slab_sb[0:C2, 0:1]

        mpsum = acc_pool.tile([C2, C2], F32, tag="m")

        chunk0 = 0
        col = 0
        first_mm = True
        for w, (nw, sq_eng, red_eng) in enumerate(waves):
            ncols = nw * C2 + (CPRE if w == 0 else 0)
            nc.sync.dma_start(
                slab_sb[:, col : col + ncols], slab[:, col : col + ncols]
            )
            col += ncols
            # Logical wave ordering for the Tile list scheduler: without
            # this, a later wave's first op can land ahead of earlier
            # waves in an engine queue and head-of-line-block on its DMA.
            ctx_w = tc.tile_wait_until(0.005 * (w + 1))
            ctx_w.__enter__()
            dw = slab_sb[:, CPRE + chunk0 * C2 : CPRE + (chunk0 + nw) * C2]
            g = 2 * nw  # column groups of 32 chans (T/S halves per chunk)

            # sq = dw^2
            sq = work.tile([128, nw * C2], F16, tag=f"sq{w}")
            if sq_eng == "act":
                nc.scalar.square(sq[:], dw)
            else:
                nc.vector.tensor_tensor(sq[:], dw, dw, op=mybir.AluOpType.mult)

            # n2[p, g] = sum over the 32 chans of group g
            if True:
                if red_eng == "dve":
                    n2 = work.tile([128, g], F16, tag=f"n2{w}")
                    nc.vector.tensor_reduce(
                        out=n2[:],
                        in_=sq[:].rearrange("p (g c) -> p g c", c=C),
                        op=mybir.AluOpType.add,
                        axis=mybir.AxisListType.X,
                    )
                else:
                    # binary fold tree 32 -> 1; all-fp16 packed last dims
                    # keep the DVE 2x mode on every level but the last
                    src = sq
                    width = C
                    lvl = 0
                    while width > 1:
                        width //= 2
                        eng = nc.gpsimd if (red_eng == "pfold" and lvl == 0) else nc.vector
                        dst = work.tile([128, g * width], F16, tag=f"f{w}_{lvl}")
                        sv = src[:].rearrange("p (g c) -> p g c", c=2 * width)
                        eng.tensor_tensor(
                            dst[:].rearrange("p (g c) -> p g c", c=width),
                            sv[:, :, 0:width],
                            sv[:, :, width : 2 * width],
                            op=mybir.AluOpType.add,
                        )
                        src = dst
                        lvl += 1
                    n2 = src

            # r2[p, g, 0:2] = 1/sqrt(n2[p, g]), written in pairs so the
            # normalize mult's last AP dim stays packed -> DVE 2x mode
            r2 = work.tile([128, 2 * g], F16, tag=f"r2{w}")
            nc.scalar.activation(
                r2[:].rearrange("p (g o) -> p g o", o=2),
                n2[:].unsqueeze(2).broadcast_to((128, g, 2)),
                ARS,
            )

            # vts[p, g, k, o] = dw[p, g, k, o] * r2[p, g, o]
            vts = work.tile([128, nw * C2], F16, tag=f"vts{w}")
            nc.vector.tensor_tensor(
                vts[:].rearrange("p (g k o) -> p g k o", k=C // 2, o=2),
                dw.rearrange("p (g k o) -> p g k o", k=C // 2, o=2),
                r2[:]
                .rearrange("p (g o) -> p g o", o=2)
                .unsqueeze(2)
                .broadcast_to((128, g, C // 2, 2)),
                op=mybir.AluOpType.mult,
            )

            # M += vts_j^T @ vts_j per chunk
            for j in range(nw):
                nc.tensor.matmul(
                    mpsum[:],
                    vts[:, bass.ts(j, C2)],
                    vts[:, bass.ts(j, C2)],
                    start=first_mm,
                    stop=(w == len(waves) - 1 and j == nw - 1),
                )
                first_mm = False
            chunk0 += nw
            ctx_w.__exit__(None, None, None)

        # Epilogue: row[j] = sum_i sgn_i * M_ij^2, shipped as [1, 64];
        # the host applies sgn_j, SCALE, and the cross-core sum.
        msq = work.tile([C2, C2], F16, tag="msq")
        nc.scalar.square(msq[:], mpsum[:])
        row_ps = acc_pool.tile([1, C2], F32, tag="row")
        nc.tensor.matmul(row_ps[:], sgn, msq[:], start=True, stop=True)
        row_sb = work.tile([1, C2], F32, tag="row_sb")
        nc.scalar.copy(row_sb[:], row_ps[:])
        nc.sync.dma_start(out_ap, row_sb[:])


def build_nc(compile: bool = True, waves=None) -> bass.Bass:
    nc = bacc.Bacc("TRN2", debug=False)
    slab = nc.dram_tensor(
        "slab", [128, CPRE + NCHUNK * C2], F16, kind="ExternalInput"
    ).ap()
    out = nc.dram_tensor("out", [1, C2], F32, kind="ExternalOutput").ap()
    with tile.TileContext(nc) as tc:
        _emit(tc, out, slab, waves or WAVES)
    if compile:
        nc.compile()
    return nc


_NC_CACHE: bass.Bass | None = None


def _get_nc() -> bass.Bass:
    global _NC_CACHE
    if _NC_CACHE is None:
        _NC_CACHE = build_nc()
    return _NC_CACHE


_SGN = np.concatenate([np.ones(C, np.float32), -np.ones(C, np.float32)])


def _pack(T, S):
    # [64, HW] fp32 -> [128, NCHUNK*64] fp16 position-major chunk layout:
    # slab[p, 1 + 64*c + ch] = V[ch, 128*c + p]
    V = np.concatenate([T, S], axis=0).astype(np.float16)
    Vt = V.T.reshape(NCHUNK, 128, C2).transpose(1, 0, 2).reshape(128, NCHUNK * C2)
    slab = np.empty((128, CPRE + NCHUNK * C2), dtype=np.float16)
    slab[:, 0] = 0.0
    slab[0:C2, 0] = _SGN
    slab[:, CPRE:] = Vt
    return slab


def kernel(preds_S, preds_T) -> np.ndarray:
    S = np.asarray(preds_S, dtype=np.float32).reshape(B * L, C, HW)
    T = np.asarray(preds_T, dtype=np.float32).reshape(B * L, C, HW)
    in_maps = [{"slab": _pack(T[i], S[i])} for i in range(N_CORES)]
    res = run_bass_kernel_spmd(_get_nc(), in_maps, list(range(N_CORES))).results
    total = np.float64(0.0)
    for i in range(N_CORES):
        row = res[i]["out"].reshape(C2).astype(np.float64)
        total += float((row * _SGN).sum())
    return np.float32(total * SCALE)
